# revision 19
# baseline (speedup 1.0000x reference)
"""Trainium2 Bass kernel: fused multi-head causal self-attention block.

Computes, for x:(B,S,H), W_qkv:(3H,H), b_qkv:(3H,), W_out:(H,H), b_out:(H,):
    qkv = x @ W_qkv.T + b_qkv ; split into q,k,v heads (NH heads, D=H/NH)
    out = softmax(causal(q k^T / sqrt(D))) v   ; merge heads
    return out @ W_out.T + b_out

Sharding over 8 NeuronCores: DP(2 batches) x TP(4 head-groups).
Core c handles batch b=c//4, head group g=c%4 (heads 4g..4g+3).
Per-head attention outputs (stored transposed, [D,S]) are AllGather'd
within each batch group of 4 cores in per-(head, 512-token-strip)
chunks; each core then computes a disjoint 512-column slice of the
output projection, so the host does a pure concatenation.

All matmul operands are fp16 (PSUM accumulation is fp32); softmax
denominators and normalization stay fp32.
"""

import math

import numpy as np

import concourse.bass as bass
import concourse.mybir as mybir
import concourse.tile as tile
from concourse import bacc
from concourse.bass_utils import run_bass_kernel_spmd

FP = mybir.dt.float32
F16 = mybir.dt.float16

# Full-size problem constants.
B, S, H, NH = 2, 2048, 2048, 16
D = 128
NCORES = 8
GROUPS = 4                  # head-groups per batch (TP degree)
REPLICA_GROUPS = [[0, 1, 2, 3], [4, 5, 6, 7]]

SKEW = 2                    # attention inner-loop software pipeline depth
TRACE = False               # set by test harness to capture NTFF profile
LAST_EXEC_NS = None
LAST_RESULTS = None


def build_nc(s=S, h=H, nh=NH, reps=1, ag=True):
    """Build the SPMD Bass program (identical on all 8 cores)."""
    nl = nh // GROUPS           # local heads per core
    dg = nl * D                 # per-core slice of the head dim
    scale = 1.0 / math.sqrt(D)

    nc = bacc.Bacc(
        "TRN2",
        target_bir_lowering=False,
        debug=False,
        enable_asserts=False,
        num_devices=NCORES,
    )

    # ---- I/O -----------------------------------------------------------
    xT_d = nc.dram_tensor("xT", [h, s], F16, kind="ExternalInput")
    wq_d = nc.dram_tensor("wq", [h, dg], F16, kind="ExternalInput")
    wk_d = nc.dram_tensor("wk", [h, dg], F16, kind="ExternalInput")
    wv_d = nc.dram_tensor("wv", [h, dg], F16, kind="ExternalInput")
    wo_d = nc.dram_tensor("wo", [h, dg], F16, kind="ExternalInput")
    bq_d = nc.dram_tensor("bq", [128, nl], FP, kind="ExternalInput")
    bk_d = nc.dram_tensor("bk", [128, nl], FP, kind="ExternalInput")
    bv_d = nc.dram_tensor("bv", [128, dg], FP, kind="ExternalInput")
    bo_d = nc.dram_tensor("bo", [128, dg], FP, kind="ExternalInput")
    mask_d = nc.dram_tensor("mask", [128, 896], F16, kind="ExternalInput")
    ones_d = nc.dram_tensor("ones", [128, 128], F16, kind="ExternalInput")
    out_d = nc.dram_tensor("out", [s, dg], FP, kind="ExternalOutput")

    with tile.TileContext(nc) as tc:
        with tc.tile_pool(name="const", bufs=1) as constp:
            mask_sb = constp.tile([128, 896], F16)
            ones_sb = constp.tile([128, 128], F16)
            bq_sb = constp.tile([128, nl], FP)
            bk_sb = constp.tile([128, nl], FP)
            bv_sb = constp.tile([128, dg], FP)
            bo_sb = constp.tile([128, dg], FP)
            ones_sq = ones_sb[:, :]           # [128,128] lhsT: denominator+broadcast

            def load_consts():
                nc.sync.dma_start(bq_sb[:], bq_d[:])
                nc.sync.dma_start(bk_sb[:], bk_d[:])
                nc.sync.dma_start(mask_sb[:], mask_d[:])
                nc.sync.dma_start(ones_sb[:], ones_d[:])
                nc.sync.dma_start(bv_sb[:], bv_d[:])
                nc.sync.dma_start(bo_sb[:], bo_d[:])

            for _rep in range(reps):
                _emit_body(nc, tc, s, h, nh,
                           xT_d, wq_d, wk_d, wv_d, wo_d, out_d,
                           bq_sb, bk_sb, bv_sb, bo_sb,
                           mask_sb, ones_sq, scale, ag,
                           load_consts if _rep == 0 else None)

    nc.compile()
    return nc


def _emit_body(nc, tc, s, h, nh,
               xT_d, wq_d, wk_d, wv_d, wo_d, out_d,
               bq_sb, bk_sb, bv_sb, bo_sb,
               mask_sb, ones_sq, scale, ag=True, load_consts=None):
    nl = nh // GROUPS
    dg = nl * D
    hc = h // 128               # 128-row contraction chunks
    hb_n = hc // 4              # batched (4-chunk) groups
    sq = s // 512
    st_n = s // 128             # 128-row s tiles
    with tc.tile_pool(name="qkv", bufs=1) as qkvp:
        qT = [qkvp.tile([128, s], F16, tag=f"qT{t}", name=f"qT{t}") for t in range(nl)]
        kT = [qkvp.tile([128, s], F16, tag=f"kT{t}", name=f"kT{t}") for t in range(nl)]
        vv = [qkvp.tile([128, dg], F16, tag=f"v{t}", name=f"v{t}") for t in range(st_n)]

        with tc.tile_pool(name="wqkv", bufs=1) as wqkvp, \
             tc.tile_pool(name="xres", bufs=1) as xp:
            # All projection weights loaded once, alive through both A phases.
            wq_sb = [wqkvp.tile([128, 4, dg], F16, tag=f"wq{hb}", name=f"wq{hb}") for hb in range(hb_n)]
            wk_sb = [wqkvp.tile([128, 4, dg], F16, tag=f"wk{hb}", name=f"wk{hb}") for hb in range(hb_n)]
            wv_sb = [wqkvp.tile([128, 4, dg], F16, tag=f"wv{hb}", name=f"wv{hb}") for hb in range(hb_n)]
            # x resident in SBUF fp16, loaded once: [128, chunk, tokens] per hb.
            xsb = [xp.tile([128, 4, s], F16, tag=f"x{hb}", name=f"x{hb}") for hb in range(hb_n)]
            for hb in range(hb_n):
                rows = slice(512 * hb, 512 * hb + 512)
                nc.sync.dma_start(wq_sb[hb][:], wq_d[rows, :].rearrange("(c p) d -> p c d", p=128))
                nc.sync.dma_start(wk_sb[hb][:], wk_d[rows, :].rearrange("(c p) d -> p c d", p=128))
                for c in range(4):  # per-chunk so the first matmuls gate on 1MB, not 2MB
                    nc.sync.dma_start(
                        xsb[hb][:, c, :],
                        xT_d[512 * hb + 128 * c:512 * hb + 128 * c + 128, :])
                if hb == 0 and load_consts is not None:
                    load_consts()  # consts are off the critical path
                nc.sync.dma_start(wv_sb[hb][:], wv_d[rows, :].rearrange("(c p) d -> p c d", p=128))

            # ---- Phase A1: Q^T and K^T projections ------------------
            # contraction-contiguous: all 16 chunks of one output tile
            # back-to-back into one PSUM bank (no per-MM bank cycling).
            with tc.tile_pool(name="psA", bufs=1, space="PSUM") as psA:
                for strip in range(sq):
                    cs = slice(512 * strip, 512 * strip + 512)
                    pss = [psA.tile([128, 512], FP, tag=f"psqk{gi}", name=f"psqk{gi}")
                           for gi in range(2 * nl)]
                    for gi in range(2 * nl):
                        w_sb = wq_sb if gi < nl else wk_sb
                        t = gi % nl
                        for hb in range(hb_n):
                            for c in range(4):
                                hh = 4 * hb + c
                                nc.tensor.matmul(
                                    pss[gi][:],
                                    w_sb[hb][:, c, 128 * t:128 * t + 128],
                                    xsb[hb][:, c, cs],
                                    start=(hh == 0), stop=(hh == hc - 1),
                                )
                        dstT = qT if gi < nl else kT
                        bias = bq_sb if gi < nl else bk_sb
                        nc.scalar.activation(
                            dstT[t][:, cs], pss[gi][:],
                            mybir.ActivationFunctionType.Identity,
                            bias=bias[:, t:t + 1],
                        )

            # ---- Phase A2: V projection (natural [s, d] layout) -----
            with tc.tile_pool(name="psV", bufs=2, space="PSUM") as psV:
                for strip in range(sq):
                    psv = [psV.tile([128, dg], FP, tag=f"psv{sti}", name=f"psv{sti}")
                           for sti in range(4)]
                    for sti in range(4):
                        ts = slice(512 * strip + 128 * sti, 512 * strip + 128 * sti + 128)
                        for hb in range(hb_n):
                            for c in range(4):
                                hh = 4 * hb + c
                                nc.tensor.matmul(
                                    psv[sti][:],
                                    xsb[hb][:, c, ts],
                                    wv_sb[hb][:, c, :],
                                    start=(hh == 0), stop=(hh == hc - 1),
                                )
                        nc.vector.tensor_add(vv[4 * strip + sti][:], psv[sti][:], bv_sb[:])

        # ---- Phase B + C: attention, chunked AllGather, overlapped out-proj
        with tc.tile_pool(name="wop", bufs=1) as wop, \
             tc.tile_pool(name="etp", bufs=8) as etp, \
             tc.tile_pool(name="atp", bufs=3) as atp, \
             tc.tile_pool(name="rbp", bufs=3) as rbp, \
             tc.tile_pool(name="oaccp", bufs=1) as oaccp, \
             tc.tile_pool(name="atsp", bufs=4) as atsp, \
             tc.tile_pool(name="outp", bufs=2) as outp, \
             tc.tile_pool(name="dramp", bufs=1, space="DRAM") as dramp, \
             tc.tile_pool(name="psS", bufs=2, space="PSUM") as psS, \
             tc.tile_pool(name="psAV", bufs=2, space="PSUM") as psAV, \
             tc.tile_pool(name="psDN", bufs=2, space="PSUM") as psDN, \
             tc.tile_pool(name="psO", bufs=2, space="PSUM") as psO:
            _emit_attention(nc, tc, s, nl, dg, sq, st_n, scale, ag,
                            qT, kT, vv, mask_sb, ones_sq,
                            bo_sb, wo_d, out_d,
                            wop, etp, atp, rbp, oaccp, atsp, outp, dramp,
                            psS, psAV, psDN, psO)


def _emit_attention(nc, tc, s, nl, dg, sq, st_n, scale, ag,
                    qT, kT, vv, mask_sb, ones_sq,
                    bo_sb, wo_d, out_d,
                    wop, etp, atp, rbp, oaccp, atsp, outp, dramp,
                    psS, psAV, psDN, psO):
        oacc = [oaccp.tile([128, dg], FP, tag=f"oacc{sti}", name=f"oacc{sti}")
                for sti in range(st_n)]

        if ag:
            # Dummy collective issued at the head of the (otherwise empty)
            # gpsimd queue: pays the ~25us collective-fabric cold-start
            # during phase A instead of on the first real AllGather.
            win = dramp.tile([128, 16], F16, tag="agwarm_i", name="agwarm_i")
            wout = dramp.tile([512, 16], F16, tag="agwarm_o", name="agwarm_o")
            nc.gpsimd.collective_compute(
                "AllGather",
                mybir.AluOpType.bypass,
                replica_groups=REPLICA_GROUPS,
                ins=[win.opt()],
                outs=[wout.opt()],
            )
        agouts = {}
        wo4 = {}

        def load_wo(l):
            wo4[l] = wop.tile([128, 4, dg], F16, tag="wo", name="wo", bufs=3)
            nc.sync.dma_start(
                wo4[l][:],
                wo_d[512 * l:512 * l + 512, :].rearrange("(c p) d -> p c d", p=128))

        def att_strip(l, qs):
            """Attention for head l, q-strip qs; ends with chunked AllGather.

            Diagonal tiles (128*kt >= 512*qs) are narrowed to their causally
            valid column range [off, 512) and their mask ([128,128] band on
            GpSimd) + accumulation are deferred to the end of the strip, so
            the PE never waits on the exp->mask chain.
            """
            qb = 512 * qs
            ps_av = psAV.tile([128, 512], FP, tag="ps_av", name="ps_av")
            # denominator + partition-broadcast fused: ones[128,128]^T @ et
            # puts sum_k et[k, q] in EVERY output partition.
            ps_dn = psDN.tile([128, 512], FP, tag="ps_dn", name="ps_dn")
            nk = 4 * qs + 4
            diag = [kt for kt in range(nk) if 128 * kt >= qb]
            offd = [kt for kt in range(nk) if 128 * kt < qb]
            # full-width off-diagonal tiles first; narrowed+masked diagonal
            # tiles last (their exp->mask chain hides under the 3-matmul
            # accumulation steps in between). Accumulated regions are nested
            # decreasing, so partial-region psum accumulation stays valid.
            score_order = offd + diag
            accum_order = offd + diag

            ets = {}

            def emit_scores(kt):
                off = max(0, 128 * kt - qb)
                w = 512 - off
                ps_s = psS.tile([128, 512], FP, tag="ps_s", name="ps_s")
                nc.tensor.matmul(
                    ps_s[:, off:512],
                    kT[l][:, 128 * kt:128 * kt + 128],
                    qT[l][:, qb + off:qb + 512],
                    start=True, stop=True,
                )
                et = etp.tile([128, 512], F16, tag="et", name="et")
                nc.scalar.activation(
                    et[:, off:512], ps_s[:, off:512],
                    mybir.ActivationFunctionType.Exp,
                    scale=scale,
                )
                if 128 * kt >= qb:  # diagonal: mask the leading 128-col band
                    nc.gpsimd.tensor_mul(
                        et[:, off:off + 128], et[:, off:off + 128],
                        mask_sb[:, 384:512])
                ets[kt] = et

            first = accum_order[0]

            def emit_accum(kt):
                et = ets.pop(kt)
                off = max(0, 128 * kt - qb)
                nc.tensor.matmul(
                    ps_dn[:, off:512], ones_sq, et[:, off:512],
                    start=(kt == first), stop=(kt == accum_order[-1]),
                )
                nc.tensor.matmul(
                    ps_av[:, off:512],
                    vv[kt][:, 128 * l:128 * l + 128],
                    et[:, off:512],
                    start=(kt == first), stop=(kt == accum_order[-1]),
                )

            # software-pipelined emission: accumulation trails scoring by SKEW
            na = 0
            for i in range(nk):
                emit_scores(score_order[i])
                if i >= SKEW:
                    emit_accum(accum_order[na])
                    na += 1
            while na < nk:
                emit_accum(accum_order[na])
                na += 1

            # normalize: an = ps_av * (1/denom)
            rb_bc = rbp.tile([128, 512], FP, tag="rb_bc", name="rb_bc")
            nc.vector.reciprocal(rb_bc[:], ps_dn[:])
            an = atp.tile([128, 512], F16, tag="an", name="an")
            nc.vector.tensor_mul(an[:], ps_av[:], rb_bc[:])
            agin = dramp.tile([128, 512], F16, tag=f"agin{l}_{qs}", name=f"agin{l}_{qs}")
            nc.sync.dma_start(agin[:], an[:])
            agout = dramp.tile([512, 512], F16, tag=f"agout{l}_{qs}",
                               name=f"agout{l}_{qs}")
            if ag:
                nc.gpsimd.collective_compute(
                    "AllGather",
                    mybir.AluOpType.bypass,
                    replica_groups=REPLICA_GROUPS,
                    ins=[agin.opt()],
                    outs=[agout.opt()],
                )
            else:  # timing ablation: local copy stands in for the collective
                nc.sync.dma_start(agout[0:128, :], agin[:])
            agouts[(l, qs)] = agout

        at4s = {}

        def prefetch_at4(l, qs):
            at4 = atsp.tile([128, 4, 512], F16, tag="at4", name="at4")
            for r in range(4):  # 4 parallel DMAs across queues
                nc.sync.dma_start(
                    at4[:, r, :],
                    agouts[(l, qs)][128 * r:128 * r + 128, :])
            at4s[(l, qs)] = at4

        def op_chunk(l, qs):
            """Out-proj contribution of head l for q-strip qs (4 token tiles)."""
            last = (l == nl - 1)
            at4 = at4s.pop((l, qs))
            for sti in range(4):
                gsti = 4 * qs + sti
                rs = slice(128 * gsti, 128 * gsti + 128)
                ps_o = psO.tile([128, dg], FP, tag="ps_o", name="ps_o")
                for r in range(4):
                    nc.tensor.matmul(
                        ps_o[:], at4[:, r, 128 * sti:128 * sti + 128], wo4[l][:, r, :],
                        start=(r == 0), stop=(r == 3),
                    )
                if l == 0:
                    nc.vector.tensor_add(oacc[gsti][:], ps_o[:], bo_sb[:])
                elif not last:
                    nc.vector.tensor_add(oacc[gsti][:], ps_o[:], oacc[gsti][:])
                else:
                    ob = outp.tile([128, dg], FP, tag="ob", name="ob")
                    nc.vector.tensor_add(ob[:], ps_o[:], oacc[gsti][:])
                    nc.sync.dma_start(out_d[rs, :], ob[:])

        # pipelined schedule: out-proj chunk (l', qs') trails attention
        # strip (l, qs) by LAG strip-slots (linear index over l*sq+qs),
        # bounding the exposed tail to LAG op chunks. at4 reads run up to
        # 2 slots ahead of their op chunk.
        LAG = 2
        total = nl * sq
        next_pf = 0
        load_wo(0)
        for t in range(total + LAG):
            if t < total:
                l, qs = divmod(t, sq)
                if qs == 0 and l + 1 < nl:
                    load_wo(l + 1)
                att_strip(l, qs)
            while next_pf < min(total, t + 1) and next_pf <= t - LAG + 2:
                prefetch_at4(*divmod(next_pf, sq))
                next_pf += 1
            to = t - LAG
            if to >= 0:
                op_chunk(*divmod(to, sq))


def make_inputs(x, W_qkv, b_qkv, W_out, b_out, s=S, h=H, nh=NH):
    """Host-side sharding: per-core input dicts."""
    nl = nh // GROUPS
    dg = nl * D
    x = np.ascontiguousarray(np.asarray(x, dtype=np.float32))
    W_qkv = np.asarray(W_qkv, dtype=np.float32)
    b_qkv = np.asarray(b_qkv, dtype=np.float32)
    W_out = np.asarray(W_out, dtype=np.float32)
    b_out = np.asarray(b_out, dtype=np.float32)

    # causal staircase master mask: mask[i, u] = 1 iff u >= i + 384
    uu = np.arange(896)[None, :]
    ii = np.arange(128)[:, None]
    mask = (uu >= ii + 384).astype(np.float16)
    ones = np.ones((128, 128), dtype=np.float16)

    WoT = W_out.T  # [h (d-in), h (n-out)]
    in_maps = []
    for c in range(NCORES):
        b, g = divmod(c, GROUPS)
        xT = np.ascontiguousarray(x[b].T.astype(np.float16))     # [h, s]
        wq = np.ascontiguousarray(W_qkv[dg * g:dg * (g + 1), :].T.astype(np.float16))
        wk = np.ascontiguousarray(W_qkv[h + dg * g:h + dg * (g + 1), :].T.astype(np.float16))
        wv = np.ascontiguousarray(W_qkv[2 * h + dg * g:2 * h + dg * (g + 1), :].T.astype(np.float16))
        bq = np.ascontiguousarray(
            b_qkv[dg * g:dg * (g + 1)].reshape(nl, 128).T)      # [128, nl]
        bk = np.ascontiguousarray(
            b_qkv[h + dg * g:h + dg * (g + 1)].reshape(nl, 128).T)
        bv = np.tile(b_qkv[2 * h + dg * g:2 * h + dg * (g + 1)][None, :], (128, 1))
        bo = np.tile(b_out[dg * g:dg * (g + 1)][None, :], (128, 1))
        # W_out^T rows permuted to the AllGather d-order:
        # ci = l*4 + r  ->  global head 4r + l (within this batch group)
        blocks = []
        for l in range(nl):
            for r in range(GROUPS):
                hh = nl * r + l  # head held as local-head l by group-rank r
                blocks.append(WoT[D * hh:D * (hh + 1), dg * g:dg * (g + 1)])
        wo = np.ascontiguousarray(
            np.concatenate(blocks, axis=0).astype(np.float16))  # [h, dg] fp16
        in_maps.append({
            "xT": xT, "wq": wq, "wk": wk, "wv": wv, "wo": wo,
            "bq": bq, "bk": bk,
            "bv": np.ascontiguousarray(bv), "bo": np.ascontiguousarray(bo),
            "mask": mask, "ones": ones,
        })
    return in_maps


_NC_CACHE = {}


def _get_nc(key=(S, H, NH)):
    if key not in _NC_CACHE:
        _NC_CACHE[key] = build_nc(*key)
    return _NC_CACHE[key]


def kernel(x, W_qkv, b_qkv, W_out, b_out):
    global LAST_EXEC_NS, LAST_RESULTS
    nc = _get_nc()
    in_maps = make_inputs(x, W_qkv, b_qkv, W_out, b_out)
    res = run_bass_kernel_spmd(
        nc, in_maps, core_ids=list(range(NCORES)), trace=TRACE)
    LAST_EXEC_NS = res.exec_time_ns
    LAST_RESULTS = res
    nl = NH // GROUPS
    dg = nl * D
    out = np.empty((B, S, H), dtype=np.float32)
    for c in range(NCORES):
        b, g = divmod(c, GROUPS)
        out[b, :, dg * g:dg * (g + 1)] = res.results[c]["out"]
    return out


# revision 21
# speedup vs baseline: 1.0359x; 1.0359x over previous
"""Trainium2 Bass kernel: fused multi-head causal self-attention block.

Computes, for x:(B,S,H), W_qkv:(3H,H), b_qkv:(3H,), W_out:(H,H), b_out:(H,):
    qkv = x @ W_qkv.T + b_qkv ; split into q,k,v heads (NH heads, D=H/NH)
    out = softmax(causal(q k^T / sqrt(D))) v   ; merge heads
    return out @ W_out.T + b_out

Sharding over 8 NeuronCores: DP(2 batches) x TP(4 head-groups).
Core c handles batch b=c//4, head group g=c%4 (heads 4g..4g+3).
Per-head attention outputs (stored transposed, [D,S]) are AllGather'd
within each batch group of 4 cores in per-(head, 512-token-strip)
chunks; each core then computes a disjoint 512-column slice of the
output projection, so the host does a pure concatenation.

All matmul operands are fp16 (PSUM accumulation is fp32); softmax
denominators and normalization stay fp32.
"""

import math

import numpy as np

import concourse.bass as bass
import concourse.mybir as mybir
import concourse.tile as tile
from concourse import bacc
from concourse.bass_utils import run_bass_kernel_spmd

FP = mybir.dt.float32
F16 = mybir.dt.float16

# Full-size problem constants.
B, S, H, NH = 2, 2048, 2048, 16
D = 128
NCORES = 8
GROUPS = 4                  # head-groups per batch (TP degree)
REPLICA_GROUPS = [[0, 1, 2, 3], [4, 5, 6, 7]]

SKEW = 2                    # attention inner-loop software pipeline depth
TRACE = False               # set by test harness to capture NTFF profile
LAST_EXEC_NS = None
LAST_RESULTS = None


def build_nc(s=S, h=H, nh=NH, reps=1, ag=True):
    """Build the SPMD Bass program (identical on all 8 cores)."""
    nl = nh // GROUPS           # local heads per core
    dg = nl * D                 # per-core slice of the head dim
    scale = 1.0 / math.sqrt(D)

    nc = bacc.Bacc(
        "TRN2",
        target_bir_lowering=False,
        debug=False,
        enable_asserts=False,
        num_devices=NCORES,
    )

    # ---- I/O -----------------------------------------------------------
    xT_d = nc.dram_tensor("xT", [h, s], F16, kind="ExternalInput")
    wq_d = nc.dram_tensor("wq", [h, dg], F16, kind="ExternalInput")
    wk_d = nc.dram_tensor("wk", [h, dg], F16, kind="ExternalInput")
    wv_d = nc.dram_tensor("wv", [h, dg], F16, kind="ExternalInput")
    wo_d = nc.dram_tensor("wo", [h, dg], F16, kind="ExternalInput")
    bq_d = nc.dram_tensor("bq", [128, nl], FP, kind="ExternalInput")
    bk_d = nc.dram_tensor("bk", [128, nl], FP, kind="ExternalInput")
    bv_d = nc.dram_tensor("bv", [128, dg], FP, kind="ExternalInput")
    bo_d = nc.dram_tensor("bo", [128, dg], FP, kind="ExternalInput")
    mask_d = nc.dram_tensor("mask", [128, 896], F16, kind="ExternalInput")
    ones_d = nc.dram_tensor("ones", [128, 128], F16, kind="ExternalInput")
    out_d = nc.dram_tensor("out", [s, dg], FP, kind="ExternalOutput")

    with tile.TileContext(nc) as tc:
        with tc.tile_pool(name="const", bufs=1) as constp:
            mask_sb = constp.tile([128, 896], F16)
            ones_sb = constp.tile([128, 128], F16)
            bq_sb = constp.tile([128, nl], FP)
            bk_sb = constp.tile([128, nl], FP)
            bv_sb = constp.tile([128, dg], FP)
            bo_sb = constp.tile([128, dg], FP)
            ones_sq = ones_sb[:, :]           # [128,128] lhsT: denominator+broadcast

            nc.sync.dma_start(bq_sb[:], bq_d[:])
            nc.sync.dma_start(bk_sb[:], bk_d[:])
            nc.sync.dma_start(mask_sb[:], mask_d[:])
            nc.sync.dma_start(ones_sb[:], ones_d[:])
            nc.sync.dma_start(bv_sb[:], bv_d[:])
            nc.sync.dma_start(bo_sb[:], bo_d[:])

            for _rep in range(reps):
                _emit_body(nc, tc, s, h, nh,
                           xT_d, wq_d, wk_d, wv_d, wo_d, out_d,
                           bq_sb, bk_sb, bv_sb, bo_sb,
                           mask_sb, ones_sq, scale, ag)

    nc.compile()
    return nc


def _emit_body(nc, tc, s, h, nh,
               xT_d, wq_d, wk_d, wv_d, wo_d, out_d,
               bq_sb, bk_sb, bv_sb, bo_sb,
               mask_sb, ones_sq, scale, ag=True):
    nl = nh // GROUPS
    dg = nl * D
    hc = h // 128               # 128-row contraction chunks
    hb_n = hc // 4              # batched (4-chunk) groups
    sq = s // 512
    st_n = s // 128             # 128-row s tiles
    with tc.tile_pool(name="qkv", bufs=1) as qkvp:
        qT = [qkvp.tile([128, s], F16, tag=f"qT{t}", name=f"qT{t}") for t in range(nl)]
        kT = [qkvp.tile([128, s], F16, tag=f"kT{t}", name=f"kT{t}") for t in range(nl)]
        vv = [qkvp.tile([128, dg], F16, tag=f"v{t}", name=f"v{t}") for t in range(st_n)]

        with tc.tile_pool(name="wqkv", bufs=1) as wqkvp, \
             tc.tile_pool(name="xres", bufs=1) as xp:
            # All projection weights loaded once, alive through both A phases.
            wq_sb = [wqkvp.tile([128, 4, dg], F16, tag=f"wq{hb}", name=f"wq{hb}") for hb in range(hb_n)]
            wk_sb = [wqkvp.tile([128, 4, dg], F16, tag=f"wk{hb}", name=f"wk{hb}") for hb in range(hb_n)]
            wv_sb = [wqkvp.tile([128, 4, dg], F16, tag=f"wv{hb}", name=f"wv{hb}") for hb in range(hb_n)]
            # x resident in SBUF fp16, loaded once: [128, chunk, tokens] per hb.
            xsb = [xp.tile([128, 4, s], F16, tag=f"x{hb}", name=f"x{hb}") for hb in range(hb_n)]
            for hb in range(hb_n):
                rows = slice(512 * hb, 512 * hb + 512)
                nc.sync.dma_start(wq_sb[hb][:], wq_d[rows, :].rearrange("(c p) d -> p c d", p=128))
                nc.sync.dma_start(wk_sb[hb][:], wk_d[rows, :].rearrange("(c p) d -> p c d", p=128))
                for c in range(4):  # per-chunk so the first matmuls gate on 1MB, not 2MB
                    nc.sync.dma_start(
                        xsb[hb][:, c, :],
                        xT_d[512 * hb + 128 * c:512 * hb + 128 * c + 128, :])
                nc.sync.dma_start(wv_sb[hb][:], wv_d[rows, :].rearrange("(c p) d -> p c d", p=128))

            # ---- Phase A1: Q^T and K^T projections ------------------
            # contraction-contiguous: all 16 chunks of one output tile
            # back-to-back into one PSUM bank (no per-MM bank cycling).
            with tc.tile_pool(name="psA", bufs=1, space="PSUM") as psA:
                for strip in range(sq):
                    cs = slice(512 * strip, 512 * strip + 512)
                    pss = [psA.tile([128, 512], FP, tag=f"psqk{gi}", name=f"psqk{gi}")
                           for gi in range(2 * nl)]
                    for gi in range(2 * nl):
                        w_sb = wq_sb if gi < nl else wk_sb
                        t = gi % nl
                        for hb in range(hb_n):
                            for c in range(4):
                                hh = 4 * hb + c
                                nc.tensor.matmul(
                                    pss[gi][:],
                                    w_sb[hb][:, c, 128 * t:128 * t + 128],
                                    xsb[hb][:, c, cs],
                                    start=(hh == 0), stop=(hh == hc - 1),
                                )
                        dstT = qT if gi < nl else kT
                        bias = bq_sb if gi < nl else bk_sb
                        nc.scalar.activation(
                            dstT[t][:, cs], pss[gi][:],
                            mybir.ActivationFunctionType.Identity,
                            bias=bias[:, t:t + 1],
                        )

            # ---- Phase A2: V projection (natural [s, d] layout) -----
            with tc.tile_pool(name="psV", bufs=2, space="PSUM") as psV:
                for strip in range(sq):
                    psv = [psV.tile([128, dg], FP, tag=f"psv{sti}", name=f"psv{sti}")
                           for sti in range(4)]
                    for sti in range(4):
                        ts = slice(512 * strip + 128 * sti, 512 * strip + 128 * sti + 128)
                        for hb in range(hb_n):
                            for c in range(4):
                                hh = 4 * hb + c
                                nc.tensor.matmul(
                                    psv[sti][:],
                                    xsb[hb][:, c, ts],
                                    wv_sb[hb][:, c, :],
                                    start=(hh == 0), stop=(hh == hc - 1),
                                )
                        nc.vector.tensor_add(vv[4 * strip + sti][:], psv[sti][:], bv_sb[:])

        # ---- Phase B + C: attention, chunked AllGather, overlapped out-proj
        with tc.tile_pool(name="wop", bufs=1) as wop, \
             tc.tile_pool(name="etp", bufs=8) as etp, \
             tc.tile_pool(name="atp", bufs=3) as atp, \
             tc.tile_pool(name="rbp", bufs=3) as rbp, \
             tc.tile_pool(name="oaccp", bufs=1) as oaccp, \
             tc.tile_pool(name="atsp", bufs=4) as atsp, \
             tc.tile_pool(name="outp", bufs=2) as outp, \
             tc.tile_pool(name="dramp", bufs=1, space="DRAM") as dramp, \
             tc.tile_pool(name="psS", bufs=2, space="PSUM") as psS, \
             tc.tile_pool(name="psAV", bufs=2, space="PSUM") as psAV, \
             tc.tile_pool(name="psDN", bufs=2, space="PSUM") as psDN, \
             tc.tile_pool(name="psO", bufs=2, space="PSUM") as psO:
            _emit_attention(nc, tc, s, nl, dg, sq, st_n, scale, ag,
                            qT, kT, vv, mask_sb, ones_sq,
                            bo_sb, wo_d, out_d,
                            wop, etp, atp, rbp, oaccp, atsp, outp, dramp,
                            psS, psAV, psDN, psO)


def _emit_attention(nc, tc, s, nl, dg, sq, st_n, scale, ag,
                    qT, kT, vv, mask_sb, ones_sq,
                    bo_sb, wo_d, out_d,
                    wop, etp, atp, rbp, oaccp, atsp, outp, dramp,
                    psS, psAV, psDN, psO):
        oacc = [oaccp.tile([128, dg], FP, tag=f"oacc{sti}", name=f"oacc{sti}")
                for sti in range(st_n)]

        if ag:
            # Dummy collective issued at the head of the (otherwise empty)
            # gpsimd queue: pays the ~25us collective-fabric cold-start
            # during phase A instead of on the first real AllGather.
            win = dramp.tile([128, 16], F16, tag="agwarm_i", name="agwarm_i")
            wout = dramp.tile([512, 16], F16, tag="agwarm_o", name="agwarm_o")
            nc.gpsimd.collective_compute(
                "AllGather",
                mybir.AluOpType.bypass,
                replica_groups=REPLICA_GROUPS,
                ins=[win.opt()],
                outs=[wout.opt()],
            )
        agouts = {}
        wo4 = {}

        def load_wo(l):
            wo4[l] = wop.tile([128, 4, dg], F16, tag="wo", name="wo", bufs=3)
            nc.sync.dma_start(
                wo4[l][:],
                wo_d[512 * l:512 * l + 512, :].rearrange("(c p) d -> p c d", p=128))

        def att_strip(l, qs):
            """Attention for head l, q-strip qs; ends with chunked AllGather.

            Diagonal tiles (128*kt >= 512*qs) are narrowed to their causally
            valid column range [off, 512) and their mask ([128,128] band on
            GpSimd) + accumulation are deferred to the end of the strip, so
            the PE never waits on the exp->mask chain.
            """
            qb = 512 * qs
            ps_av = psAV.tile([128, 512], FP, tag="ps_av", name="ps_av")
            # denominator + partition-broadcast fused: ones[128,128]^T @ et
            # puts sum_k et[k, q] in EVERY output partition.
            ps_dn = psDN.tile([128, 512], FP, tag="ps_dn", name="ps_dn")
            nk = 4 * qs + 4
            diag = [kt for kt in range(nk) if 128 * kt >= qb]
            offd = [kt for kt in range(nk) if 128 * kt < qb]
            # full-width off-diagonal tiles first; narrowed+masked diagonal
            # tiles last (their exp->mask chain hides under the 3-matmul
            # accumulation steps in between). Accumulated regions are nested
            # decreasing, so partial-region psum accumulation stays valid.
            score_order = offd + diag
            accum_order = offd + diag

            ets = {}

            def emit_scores(kt):
                off = max(0, 128 * kt - qb)
                w = 512 - off
                ps_s = psS.tile([128, 512], FP, tag="ps_s", name="ps_s")
                nc.tensor.matmul(
                    ps_s[:, off:512],
                    kT[l][:, 128 * kt:128 * kt + 128],
                    qT[l][:, qb + off:qb + 512],
                    start=True, stop=True,
                )
                et = etp.tile([128, 512], F16, tag="et", name="et")
                nc.scalar.activation(
                    et[:, off:512], ps_s[:, off:512],
                    mybir.ActivationFunctionType.Exp,
                    scale=scale,
                )
                if 128 * kt >= qb:  # diagonal: mask the leading 128-col band
                    nc.vector.tensor_mul(
                        et[:, off:off + 128], et[:, off:off + 128],
                        mask_sb[:, 384:512])
                ets[kt] = et

            first = accum_order[0]

            def emit_accum(kt):
                et = ets.pop(kt)
                off = max(0, 128 * kt - qb)
                nc.tensor.matmul(
                    ps_dn[:, off:512], ones_sq, et[:, off:512],
                    start=(kt == first), stop=(kt == accum_order[-1]),
                )
                nc.tensor.matmul(
                    ps_av[:, off:512],
                    vv[kt][:, 128 * l:128 * l + 128],
                    et[:, off:512],
                    start=(kt == first), stop=(kt == accum_order[-1]),
                )

            # software-pipelined emission: accumulation trails scoring by SKEW
            na = 0
            for i in range(nk):
                emit_scores(score_order[i])
                if i >= SKEW:
                    emit_accum(accum_order[na])
                    na += 1
            while na < nk:
                emit_accum(accum_order[na])
                na += 1

            # normalize: an = ps_av * (1/denom)
            rb_bc = rbp.tile([128, 512], FP, tag="rb_bc", name="rb_bc")
            nc.vector.reciprocal(rb_bc[:], ps_dn[:])
            an = atp.tile([128, 512], F16, tag="an", name="an")
            nc.vector.tensor_mul(an[:], ps_av[:], rb_bc[:])
            agin = dramp.tile([128, 512], F16, tag=f"agin{l}_{qs}", name=f"agin{l}_{qs}")
            nc.sync.dma_start(agin[:], an[:])
            agout = dramp.tile([512, 512], F16, tag=f"agout{l}_{qs}",
                               name=f"agout{l}_{qs}")
            if ag:
                nc.gpsimd.collective_compute(
                    "AllGather",
                    mybir.AluOpType.bypass,
                    replica_groups=REPLICA_GROUPS,
                    ins=[agin.opt()],
                    outs=[agout.opt()],
                )
            else:  # timing ablation: local copy stands in for the collective
                nc.sync.dma_start(agout[0:128, :], agin[:])
            agouts[(l, qs)] = agout

        at4s = {}

        def prefetch_at4(l, qs):
            at4 = atsp.tile([128, 4, 512], F16, tag="at4", name="at4")
            for r in range(4):  # 4 parallel DMAs across queues
                nc.sync.dma_start(
                    at4[:, r, :],
                    agouts[(l, qs)][128 * r:128 * r + 128, :])
            at4s[(l, qs)] = at4

        def op_chunk(l, qs):
            """Out-proj contribution of head l for q-strip qs (4 token tiles)."""
            last = (l == nl - 1)
            at4 = at4s.pop((l, qs))
            for sti in range(4):
                gsti = 4 * qs + sti
                rs = slice(128 * gsti, 128 * gsti + 128)
                ps_o = psO.tile([128, dg], FP, tag="ps_o", name="ps_o")
                for r in range(4):
                    nc.tensor.matmul(
                        ps_o[:], at4[:, r, 128 * sti:128 * sti + 128], wo4[l][:, r, :],
                        start=(r == 0), stop=(r == 3),
                    )
                if l == 0:
                    nc.vector.tensor_add(oacc[gsti][:], ps_o[:], bo_sb[:])
                elif not last:
                    nc.vector.tensor_add(oacc[gsti][:], ps_o[:], oacc[gsti][:])
                else:
                    ob = outp.tile([128, dg], FP, tag="ob", name="ob")
                    nc.vector.tensor_add(ob[:], ps_o[:], oacc[gsti][:])
                    nc.sync.dma_start(out_d[rs, :], ob[:])

        # pipelined schedule: out-proj chunk (l', qs') trails attention
        # strip (l, qs) by LAG strip-slots (linear index over l*sq+qs),
        # bounding the exposed tail to LAG op chunks. at4 reads run up to
        # 2 slots ahead of their op chunk.
        LAG = 2
        total = nl * sq
        next_pf = 0
        load_wo(0)
        for t in range(total + LAG):
            if t < total:
                l, qs = divmod(t, sq)
                if qs == 0 and l + 1 < nl:
                    load_wo(l + 1)
                att_strip(l, qs)
            while next_pf < min(total, t + 1) and next_pf <= t - LAG + 2:
                prefetch_at4(*divmod(next_pf, sq))
                next_pf += 1
            to = t - LAG
            if to >= 0:
                op_chunk(*divmod(to, sq))


def make_inputs(x, W_qkv, b_qkv, W_out, b_out, s=S, h=H, nh=NH):
    """Host-side sharding: per-core input dicts."""
    nl = nh // GROUPS
    dg = nl * D
    x = np.ascontiguousarray(np.asarray(x, dtype=np.float32))
    W_qkv = np.asarray(W_qkv, dtype=np.float32)
    b_qkv = np.asarray(b_qkv, dtype=np.float32)
    W_out = np.asarray(W_out, dtype=np.float32)
    b_out = np.asarray(b_out, dtype=np.float32)

    # causal staircase master mask: mask[i, u] = 1 iff u >= i + 384
    uu = np.arange(896)[None, :]
    ii = np.arange(128)[:, None]
    mask = (uu >= ii + 384).astype(np.float16)
    ones = np.ones((128, 128), dtype=np.float16)

    WoT = W_out.T  # [h (d-in), h (n-out)]
    in_maps = []
    for c in range(NCORES):
        b, g = divmod(c, GROUPS)
        xT = np.ascontiguousarray(x[b].T.astype(np.float16))     # [h, s]
        wq = np.ascontiguousarray(W_qkv[dg * g:dg * (g + 1), :].T.astype(np.float16))
        wk = np.ascontiguousarray(W_qkv[h + dg * g:h + dg * (g + 1), :].T.astype(np.float16))
        wv = np.ascontiguousarray(W_qkv[2 * h + dg * g:2 * h + dg * (g + 1), :].T.astype(np.float16))
        bq = np.ascontiguousarray(
            b_qkv[dg * g:dg * (g + 1)].reshape(nl, 128).T)      # [128, nl]
        bk = np.ascontiguousarray(
            b_qkv[h + dg * g:h + dg * (g + 1)].reshape(nl, 128).T)
        bv = np.tile(b_qkv[2 * h + dg * g:2 * h + dg * (g + 1)][None, :], (128, 1))
        bo = np.tile(b_out[dg * g:dg * (g + 1)][None, :], (128, 1))
        # W_out^T rows permuted to the AllGather d-order:
        # ci = l*4 + r  ->  global head 4r + l (within this batch group)
        blocks = []
        for l in range(nl):
            for r in range(GROUPS):
                hh = nl * r + l  # head held as local-head l by group-rank r
                blocks.append(WoT[D * hh:D * (hh + 1), dg * g:dg * (g + 1)])
        wo = np.ascontiguousarray(
            np.concatenate(blocks, axis=0).astype(np.float16))  # [h, dg] fp16
        in_maps.append({
            "xT": xT, "wq": wq, "wk": wk, "wv": wv, "wo": wo,
            "bq": bq, "bk": bk,
            "bv": np.ascontiguousarray(bv), "bo": np.ascontiguousarray(bo),
            "mask": mask, "ones": ones,
        })
    return in_maps


_NC_CACHE = {}


def _get_nc(key=(S, H, NH)):
    if key not in _NC_CACHE:
        _NC_CACHE[key] = build_nc(*key)
    return _NC_CACHE[key]


def kernel(x, W_qkv, b_qkv, W_out, b_out):
    global LAST_EXEC_NS, LAST_RESULTS
    nc = _get_nc()
    in_maps = make_inputs(x, W_qkv, b_qkv, W_out, b_out)
    res = run_bass_kernel_spmd(
        nc, in_maps, core_ids=list(range(NCORES)), trace=TRACE)
    LAST_EXEC_NS = res.exec_time_ns
    LAST_RESULTS = res
    nl = NH // GROUPS
    dg = nl * D
    out = np.empty((B, S, H), dtype=np.float32)
    for c in range(NCORES):
        b, g = divmod(c, GROUPS)
        out[b, :, dg * g:dg * (g + 1)] = res.results[c]["out"]
    return out
